# revision 17
# baseline (speedup 1.0000x reference)
"""CredalAttention Trainium2 kernel.

Reference computation (fp32, per head h of H=16, batch n of N=2):
    q = query @ Wq.T + bq ; k = key @ Wk.T + bk ; v = value @ Wv.T + bv
    scores = q k^T / sqrt(64)                  (L, S) per (n,h)
    m = rowmax(scores); ev = exp(scores - m); alpha = ev + 1
    total = sum_s alpha = sum_s ev + S
    attn = alpha / (total + 1e-10)             -> output (N,H,L,S)
    u = S / (total + 1e-10); avg over heads    -> output (N,L)
    out = concat_h(attn @ v) @ Wo.T + bo       -> output (L,N,E)

Sharding: 8 cores x 2 heads (tensor parallel over H). Each core projects
Q/K/V for its 2 heads from the full (bf16-cast) activations, computes
scores/attn in (l, s) layout (contiguous S rows for the 512MB attn-weights
write), and multiplies its 128 head-columns into out_proj columns ->
partial (L*N, E) output summed across cores on the host (+ bias).

Performance structure:
  - matmuls in bf16 (fp32-family matmuls measure ~4 cyc/row on HW);
    softmax math and the attn-weights/uncertainty outputs stay fp32.
  - exp runs straight off PSUM (scores/8 <= ~6 so exp can't overflow);
    the row max is taken AFTER exp on SBUF and folded in as
    e^-m = 1/rowmax(exp), so PSUM turns over immediately and the PE
    pipeline stays dense (HAM stays warm).
  - all (l,s)->(s,l) transposes go through dma_start_transpose (bf16),
    not the PE: inputs are transpose-loaded from DRAM, attn tiles are
    XBAR-transposed SBUF->SBUF for the attn @ V matmul.
"""

from contextlib import ExitStack

import ml_dtypes
import numpy as np

import concourse.bacc as bacc
import concourse.mybir as mybir
import concourse.tile as tile
from concourse.masks import make_identity

F32 = mybir.dt.float32
BF16 = mybir.dt.bfloat16
AX = mybir.AxisListType.X
MULT = mybir.AluOpType.mult
ADD = mybir.AluOpType.add
EXP = mybir.ActivationFunctionType.Exp
COPY = mybir.ActivationFunctionType.Copy
IDENT = mybir.ActivationFunctionType.Identity

E = 1024
H = 16
NB = 2  # batch
D = 64  # head dim
NCORES = 8
HPC = H // NCORES  # heads per core = 2
D2 = HPC * D  # 128 head dims per core


def build_nc(L=2048, S=2048):
    """Build the per-core SPMD Bass program (same program on all 8 cores;
    per-core weight slices arrive as inputs)."""
    assert L % 256 == 0 and S % 256 == 0
    TQ = L * NB
    W = 1024 if S % 1024 == 0 else 256  # score-tile width
    nW = S // W
    LG = 512 if L % 512 == 0 else 256  # l-group per attn@V round
    NH = LG // 128
    nLC = L // 128
    nSC = S // 128

    nc = bacc.Bacc("TRN2", target_bir_lowering=False, debug=False)

    xq = nc.dram_tensor("xq", [TQ, E], BF16, kind="ExternalInput")
    xk = nc.dram_tensor("xk", [S * NB, E], BF16, kind="ExternalInput")
    xv = nc.dram_tensor("xv", [S * NB, E], BF16, kind="ExternalInput")
    wqT = nc.dram_tensor("wqT", [E, D2], BF16, kind="ExternalInput")
    wkT = nc.dram_tensor("wkT", [E, D2], BF16, kind="ExternalInput")
    wvT = nc.dram_tensor("wvT", [E, D2], BF16, kind="ExternalInput")
    bqd = nc.dram_tensor("bq", [D2, 1], F32, kind="ExternalInput")
    bkd = nc.dram_tensor("bk", [D2, 1], F32, kind="ExternalInput")
    bvd = nc.dram_tensor("bv", [D2, 1], F32, kind="ExternalInput")
    wcT = nc.dram_tensor("wcT", [D2, E], BF16, kind="ExternalInput")

    attn_o = nc.dram_tensor("attn", [2 * HPC, L, S], F32, kind="ExternalOutput")
    outp_o = nc.dram_tensor("outp", [TQ, E], F32, kind="ExternalOutput")
    unc_o = nc.dram_tensor("unc", [128, 2 * HPC, nLC], F32, kind="ExternalOutput")

    with tile.TileContext(nc) as tc, ExitStack() as ctx:
        const = ctx.enter_context(tc.tile_pool(name="const", bufs=1))
        pers = ctx.enter_context(tc.tile_pool(name="pers", bufs=1))
        stage = ctx.enter_context(tc.tile_pool(name="stage", bufs=3))
        ev_pool = ctx.enter_context(tc.tile_pool(name="evp", bufs=6))
        evb_pool = ctx.enter_context(tc.tile_pool(name="evbp", bufs=4))
        evt_pool = ctx.enter_context(tc.tile_pool(name="evtp", bufs=2))
        out_pool = ctx.enter_context(tc.tile_pool(name="outp", bufs=3))
        small = ctx.enter_context(tc.tile_pool(name="small", bufs=8))
        big_ps = ctx.enter_context(tc.tile_pool(name="bigps", bufs=2, space="PSUM"))
        mid_ps = ctx.enter_context(tc.tile_pool(name="midps", bufs=2, space="PSUM"))
        ets_ps = ctx.enter_context(tc.tile_pool(name="etsps", bufs=2, space="PSUM"))

        ident = const.tile([128, 128], BF16, tag="ident")
        make_identity(nc, ident)

        # --- per-core weight slices ---
        wts = {}
        for name, dram in (("q", wqT), ("k", wkT), ("v", wvT)):
            t = const.tile([128, E // 128, D2], BF16, tag=f"w{name}T")
            nc.sync.dma_start(t[:], dram.rearrange("(eo ei) d -> ei eo d", ei=128))
            wts[name] = t
        bias = {}
        for name, dram in (("q", bqd), ("k", bkd), ("v", bvd)):
            t = const.tile([D2, 1], F32, tag=f"b{name}")
            nc.sync.dma_start(t[:], dram[:])
            bias[name] = t
        wcT_t = const.tile([D2, E], BF16, tag="wcT")
        nc.sync.dma_start(wcT_t[:], wcT[:])

        # --- persistent activations ---
        qTs = pers.tile([D2, NB, L], BF16, tag="qTs")
        kTs = pers.tile([D2, NB, S], BF16, tag="kTs")
        vTs = pers.tile([D2, NB, S], BF16, tag="vTs")
        Vs = pers.tile([128, NB, nSC, D2], BF16, tag="Vs")
        AT = pers.tile([D2, NB, L], BF16, tag="AT")
        unc_t = pers.tile([128, 2 * HPC, nLC], F32, tag="unc")

        # --- phase B: transpose-load inputs + project Q/K/V ---
        nE = E // 128
        for xdram, wname, dst in ((xq, "q", qTs), (xk, "k", kTs), (xv, "v", vTs)):
            T = xdram.shape[0]
            for g2 in range(T // 512):
                xts = stage.tile([128, nE, 512], BF16, tag="xts")
                nc.sync.dma_start_transpose(
                    xts[:], xdram[g2 * 512 : (g2 + 1) * 512, :]
                )
                psq = mid_ps.tile([128, 512], F32, tag="mid")
                for eo in range(nE):
                    nc.tensor.matmul(
                        psq[:D2, :],
                        wts[wname][:, eo, :],
                        xts[:, eo, :],
                        start=(eo == 0),
                        stop=(eo == nE - 1),
                    )
                # de-interleave batch (token t = l*NB + n) + bias via ACT copy
                ps_v = psq.rearrange("d (l n) -> d n l", n=NB)
                for n in range(NB):
                    nc.scalar.activation(
                        dst[:, n, g2 * 256 : (g2 + 1) * 256],
                        ps_v[:D2, n, :],
                        IDENT,
                        bias=bias[wname],
                    )

        # --- phase B2: V^T (d,s) -> V (s,d) for the attn @ V matmul ---
        for n in range(NB):
            nc.sync.dma_start_transpose(Vs[:, n, :, :], vTs[:, n, :])

        outp_v = outp_o.rearrange("(l n) e -> l n e", n=NB)

        def emit_out_proj(n):
            for lc in range(nLC):
                op0 = big_ps.tile([128, E], F32, tag="big")
                for j in range(E // 512):
                    nc.tensor.matmul(
                        op0[:, j * 512 : (j + 1) * 512],
                        AT[:, n, lc * 128 : (lc + 1) * 128],
                        wcT_t[:, j * 512 : (j + 1) * 512],
                        start=True,
                        stop=True,
                    )
                ot = out_pool.tile([128, E], F32, tag="ot")
                nc.vector.tensor_copy(ot[:], op0[:])
                nc.sync.dma_start(outp_v[lc * 128 : (lc + 1) * 128, n, :], ot[:])

        # --- phase C: per (batch, head) attention ---
        for p in range(NB * HPC):
            n, hh = p // HPC, p % HPC
            qh = qTs[hh * D : (hh + 1) * D, n, :]
            kh = kTs[hh * D : (hh + 1) * D, n, :]
            for lg in range(L // LG):
                evTs_t = evt_pool.tile([128, nSC, LG], BF16, tag="evts")
                ev_halves = []
                for half in range(NH):
                    lc = lg * NH + half
                    ev_t = ev_pool.tile([128, S], F32, tag="ev")
                    sums = small.tile([128, nW], F32, tag="sums")
                    mxw = small.tile([128, nW], F32, tag="mxw")
                    for w in range(nW):
                        st = big_ps.tile([128, W], F32, tag="big")
                        for j in range(max(1, W // 512)):
                            jw = min(512, W)
                            nc.tensor.matmul(
                                st[:, j * jw : (j + 1) * jw],
                                qh[:, lc * 128 : (lc + 1) * 128],
                                kh[:, w * W + j * jw : w * W + (j + 1) * jw],
                                start=True,
                                stop=True,
                            )
                        # exp straight off PSUM (scores/8 <= ~6, no overflow);
                        # raw-score row max read off PSUM in parallel and
                        # folded in later as e^-m = exp(-max/8)
                        nc.scalar.activation(
                            ev_t[:, w * W : (w + 1) * W],
                            st[:],
                            EXP,
                            bias=0.0,
                            scale=0.125,
                            accum_out=sums[:, w : w + 1],
                        )
                        nc.vector.reduce_max(mxw[:, w : w + 1], st[:], axis=AX)
                    mx = small.tile([128, 1], F32, tag="mx")
                    nc.vector.reduce_max(mx[:], mxw[:], axis=AX)
                    em = small.tile([128, 1], F32, tag="em")
                    nc.scalar.activation(em[:], mx[:], EXP, scale=-0.125)
                    stot = small.tile([128, 1], F32, tag="stot")
                    nc.vector.reduce_sum(stot[:], sums[:], axis=AX)
                    nc.gpsimd.tensor_tensor(stot[:], stot[:], em[:], op=MULT)
                    nc.gpsimd.tensor_scalar_add(stot[:], stot[:], float(S))
                    rr = small.tile([128, 1], F32, tag="rr")
                    nc.vector.reciprocal(rr[:], stot[:])
                    emr = small.tile([128, 1], F32, tag="emr")
                    nc.gpsimd.tensor_tensor(emr[:], em[:], rr[:], op=MULT)
                    nc.gpsimd.tensor_scalar_mul(
                        unc_t[:, p, lc : lc + 1], rr[:], float(S)
                    )
                    # bf16 attn for the transpose + attn @ V path (DVE, in
                    # parallel with the fp32 in-place pass on GpSimd below)
                    evb = evb_pool.tile([128, S], BF16, tag="evb")
                    nc.vector.tensor_scalar(evb[:], ev_t[:], emr[:], rr[:], MULT, ADD)
                    ev_halves.append(evb)
                    # attn = ev'*(e^-m * r) + r  (in place, fp32 for output)
                    nc.gpsimd.tensor_scalar(ev_t[:], ev_t[:], emr[:], rr[:], MULT, ADD)
                    nc.sync.dma_start(attn_o[p, lc * 128 : (lc + 1) * 128, :], ev_t[:])
                peg = 2 if nSC % 2 == 0 else 1
                for sg in range(nSC // peg):
                    ets = ets_ps.tile([128, peg, LG], BF16, tag="etsbf")
                    for j in range(peg):
                        sc = sg * peg + j
                        for half in range(NH):
                            nc.tensor.transpose(
                                ets[:, j, half * 128 : (half + 1) * 128],
                                ev_halves[half][:, sc * 128 : (sc + 1) * 128],
                                ident,
                            )
                    dst_sl = evTs_t[:, sg * peg : (sg + 1) * peg, :]
                    if sg % 2 == 0:
                        nc.vector.tensor_copy(dst_sl, ets[:])
                    else:
                        nc.scalar.activation(dst_sl, ets[:], COPY)
                avp = mid_ps.tile([128, LG], F32, tag="mid")
                for sc in range(nSC):
                    nc.tensor.matmul(
                        avp[:],
                        Vs[:, n, sc, :],
                        evTs_t[:, sc, :],
                        start=(sc == 0),
                        stop=(sc == nSC - 1),
                    )
                nc.scalar.activation(
                    AT[hh * D : (hh + 1) * D, n, lg * LG : (lg + 1) * LG],
                    avp[hh * D : (hh + 1) * D, :],
                    COPY,
                )
            if hh == HPC - 1:
                emit_out_proj(n)

        nc.sync.dma_start(unc_o[:], unc_t[:])

    nc.compile()
    return nc


_NC_CACHE = {}


def get_nc(L=2048, S=2048):
    key = (L, S)
    if key not in _NC_CACHE:
        _NC_CACHE[key] = build_nc(L, S)
    return _NC_CACHE[key]


def make_in_maps(query, key, value, in_proj_w, in_proj_b, out_proj_w):
    """Host-side sharding: slice per-core weights, flatten + bf16-cast
    activations."""
    L, N, Ei = query.shape
    S = key.shape[0]
    bf = ml_dtypes.bfloat16
    xq = np.ascontiguousarray(query.reshape(L * N, Ei)).astype(bf)
    xk = np.ascontiguousarray(key.reshape(S * N, Ei)).astype(bf)
    xv = np.ascontiguousarray(value.reshape(S * N, Ei)).astype(bf)
    in_maps = []
    for c in range(NCORES):
        r0 = c * D2
        wq = in_proj_w[r0 : r0 + D2, :]
        wk = in_proj_w[Ei + r0 : Ei + r0 + D2, :]
        wv = in_proj_w[2 * Ei + r0 : 2 * Ei + r0 + D2, :]
        in_maps.append(
            {
                "xq": xq,
                "xk": xk,
                "xv": xv,
                "wqT": np.ascontiguousarray(wq.T).astype(bf),
                "wkT": np.ascontiguousarray(wk.T).astype(bf),
                "wvT": np.ascontiguousarray(wv.T).astype(bf),
                "bq": np.ascontiguousarray(
                    in_proj_b[r0 : r0 + D2].reshape(D2, 1), dtype=np.float32
                ),
                "bk": np.ascontiguousarray(
                    in_proj_b[Ei + r0 : Ei + r0 + D2].reshape(D2, 1), dtype=np.float32
                ),
                "bv": np.ascontiguousarray(
                    in_proj_b[2 * Ei + r0 : 2 * Ei + r0 + D2].reshape(D2, 1),
                    dtype=np.float32,
                ),
                "wcT": np.ascontiguousarray(out_proj_w[:, r0 : r0 + D2].T).astype(
                    bf
                ),
            }
        )
    return in_maps


def assemble(results, out_proj_b, L, S):
    """Host-side gather: concat heads, sum out-proj partials, avg uncertainty."""
    nLC = L // 128
    attn_w = np.empty((NB, H, L, S), dtype=np.float32)
    avg_u = np.zeros((NB, L), dtype=np.float32)
    outp = np.zeros((L * NB, E), dtype=np.float32)
    for c, res in enumerate(results):
        a = res["attn"].reshape(NB, HPC, L, S)
        attn_w[:, c * HPC : (c + 1) * HPC] = a
        outp += res["outp"]
        # unc[p_inner, n*HPC+hh, lc] holds S*r for l = lc*128 + p_inner
        u = res["unc"].reshape(128, NB, HPC, nLC).sum(axis=2)  # (128, NB, nLC)
        avg_u += u.transpose(1, 2, 0).reshape(NB, L)
    avg_u /= H
    outp += out_proj_b[None, :].astype(np.float32)
    attn_out = outp.reshape(L, NB, E)
    return attn_out, attn_w, avg_u


def kernel(query, key, value, in_proj_w, in_proj_b, out_proj_w, out_proj_b):
    from concourse.bass_utils import run_bass_kernel_spmd

    L, N, Ei = query.shape
    S = key.shape[0]
    assert (N, Ei) == (NB, E)
    nc = get_nc(L, S)
    in_maps = make_in_maps(query, key, value, in_proj_w, in_proj_b, out_proj_w)
    res = run_bass_kernel_spmd(nc, in_maps, core_ids=list(range(NCORES)))
    return assemble(res.results, np.asarray(out_proj_b), L, S)


# revision 18
# speedup vs baseline: 1.1747x; 1.1747x over previous
"""CredalAttention Trainium2 kernel.

Reference computation (fp32, per head h of H=16, batch n of N=2):
    q = query @ Wq.T + bq ; k = key @ Wk.T + bk ; v = value @ Wv.T + bv
    scores = q k^T / sqrt(64)                  (L, S) per (n,h)
    m = rowmax(scores); ev = exp(scores - m); alpha = ev + 1
    total = sum_s alpha = sum_s ev + S
    attn = alpha / (total + 1e-10)             -> output (N,H,L,S)
    u = S / (total + 1e-10); avg over heads    -> output (N,L)
    out = concat_h(attn @ v) @ Wo.T + bo       -> output (L,N,E)

Sharding: 8 cores x 2 heads (tensor parallel over H). Each core projects
Q/K/V for its 2 heads from the full (bf16-cast) activations, computes
scores/attn in (l, s) layout (contiguous S rows for the 512MB attn-weights
write), and multiplies its 128 head-columns into out_proj columns ->
partial (L*N, E) output summed across cores on the host (+ bias).

Performance structure:
  - matmuls in bf16 (fp32-family matmuls measure ~4 cyc/row on HW);
    softmax math and the attn-weights/uncertainty outputs stay fp32.
  - exp runs straight off PSUM (scores/8 <= ~6 so exp can't overflow);
    the row max is taken AFTER exp on SBUF and folded in as
    e^-m = 1/rowmax(exp), so PSUM turns over immediately and the PE
    pipeline stays dense (HAM stays warm).
  - all (l,s)->(s,l) transposes go through dma_start_transpose (bf16),
    not the PE: inputs are transpose-loaded from DRAM, attn tiles are
    XBAR-transposed SBUF->SBUF for the attn @ V matmul.
"""

from contextlib import ExitStack

import ml_dtypes
import numpy as np

import concourse.bacc as bacc
import concourse.mybir as mybir
import concourse.tile as tile
from concourse.masks import make_identity

F32 = mybir.dt.float32
BF16 = mybir.dt.bfloat16
AX = mybir.AxisListType.X
MULT = mybir.AluOpType.mult
ADD = mybir.AluOpType.add
EXP = mybir.ActivationFunctionType.Exp
COPY = mybir.ActivationFunctionType.Copy
IDENT = mybir.ActivationFunctionType.Identity

E = 1024
H = 16
NB = 2  # batch
D = 64  # head dim
NCORES = 8
HPC = H // NCORES  # heads per core = 2
D2 = HPC * D  # 128 head dims per core


def build_nc(L=2048, S=2048):
    """Build the per-core SPMD Bass program (same program on all 8 cores;
    per-core weight slices arrive as inputs)."""
    assert L % 256 == 0 and S % 256 == 0
    TQ = L * NB
    W = 1024 if S % 1024 == 0 else 256  # score-tile width
    nW = S // W
    nLC = L // 128
    nSC = S // 128

    nc = bacc.Bacc("TRN2", target_bir_lowering=False, debug=False)

    xq = nc.dram_tensor("xq", [TQ, E], BF16, kind="ExternalInput")
    xk = nc.dram_tensor("xk", [S * NB, E], BF16, kind="ExternalInput")
    xv = nc.dram_tensor("xv", [S * NB, E], BF16, kind="ExternalInput")
    wqT = nc.dram_tensor("wqT", [E, D2], BF16, kind="ExternalInput")
    wkT = nc.dram_tensor("wkT", [E, D2], BF16, kind="ExternalInput")
    wvT = nc.dram_tensor("wvT", [E, D2], BF16, kind="ExternalInput")
    bqd = nc.dram_tensor("bq", [D2, 1], F32, kind="ExternalInput")
    bkd = nc.dram_tensor("bk", [D2, 1], F32, kind="ExternalInput")
    bvd = nc.dram_tensor("bv", [D2, 1], F32, kind="ExternalInput")
    wcT = nc.dram_tensor("wcT", [D2, E], BF16, kind="ExternalInput")

    attn_o = nc.dram_tensor("attn", [2 * HPC, L, S], F32, kind="ExternalOutput")
    outp_o = nc.dram_tensor("outp", [TQ, E], F32, kind="ExternalOutput")
    unc_o = nc.dram_tensor("unc", [128, 2 * HPC, nLC], F32, kind="ExternalOutput")

    with tile.TileContext(nc) as tc, ExitStack() as ctx:
        const = ctx.enter_context(tc.tile_pool(name="const", bufs=1))
        pers = ctx.enter_context(tc.tile_pool(name="pers", bufs=1))
        stage = ctx.enter_context(tc.tile_pool(name="stage", bufs=3))
        ev_pool = ctx.enter_context(tc.tile_pool(name="evp", bufs=6))
        evb_pool = ctx.enter_context(tc.tile_pool(name="evbp", bufs=4))
        evt_pool = ctx.enter_context(tc.tile_pool(name="evtp", bufs=2))
        out_pool = ctx.enter_context(tc.tile_pool(name="outp", bufs=3))
        small = ctx.enter_context(tc.tile_pool(name="small", bufs=8))
        big_ps = ctx.enter_context(tc.tile_pool(name="bigps", bufs=2, space="PSUM"))
        mid_ps = ctx.enter_context(tc.tile_pool(name="midps", bufs=2, space="PSUM"))
        ets_ps = ctx.enter_context(tc.tile_pool(name="etsps", bufs=2, space="PSUM"))

        ident = const.tile([128, 128], BF16, tag="ident")
        make_identity(nc, ident)

        # --- per-core weight slices ---
        wts = {}
        for name, dram in (("q", wqT), ("k", wkT), ("v", wvT)):
            t = const.tile([128, E // 128, D2], BF16, tag=f"w{name}T")
            nc.sync.dma_start(t[:], dram.rearrange("(eo ei) d -> ei eo d", ei=128))
            wts[name] = t
        bias = {}
        for name, dram in (("q", bqd), ("k", bkd), ("v", bvd)):
            t = const.tile([D2, 1], F32, tag=f"b{name}")
            nc.sync.dma_start(t[:], dram[:])
            bias[name] = t
        wcT_t = const.tile([D2, E], BF16, tag="wcT")
        nc.sync.dma_start(wcT_t[:], wcT[:])

        # --- persistent activations ---
        qTs = pers.tile([D2, NB, L], BF16, tag="qTs")
        kTs = pers.tile([D2, NB, S], BF16, tag="kTs")
        vTs = pers.tile([D2, NB, S], BF16, tag="vTs")
        Vs = pers.tile([128, NB, nSC, D2], BF16, tag="Vs")
        AT = pers.tile([D2, NB, L], BF16, tag="AT")
        unc_t = pers.tile([128, 2 * HPC, nLC], F32, tag="unc")

        # --- phase B: transpose-load inputs + project Q/K/V ---
        nE = E // 128
        for xdram, wname, dst in ((xq, "q", qTs), (xk, "k", kTs), (xv, "v", vTs)):
            T = xdram.shape[0]
            for g2 in range(T // 512):
                xts = stage.tile([128, nE, 512], BF16, tag="xts")
                nc.sync.dma_start_transpose(
                    xts[:], xdram[g2 * 512 : (g2 + 1) * 512, :]
                )
                psq = mid_ps.tile([128, 512], F32, tag="mid")
                for eo in range(nE):
                    nc.tensor.matmul(
                        psq[:D2, :],
                        wts[wname][:, eo, :],
                        xts[:, eo, :],
                        start=(eo == 0),
                        stop=(eo == nE - 1),
                    )
                # de-interleave batch (token t = l*NB + n) + bias via ACT copy
                ps_v = psq.rearrange("d (l n) -> d n l", n=NB)
                for n in range(NB):
                    nc.scalar.activation(
                        dst[:, n, g2 * 256 : (g2 + 1) * 256],
                        ps_v[:D2, n, :],
                        IDENT,
                        bias=bias[wname],
                    )

        # --- phase B2: V^T (d,s) -> V (s,d) for the attn @ V matmul ---
        for n in range(NB):
            nc.sync.dma_start_transpose(Vs[:, n, :, :], vTs[:, n, :])

        outp_v = outp_o.rearrange("(l n) e -> l n e", n=NB)

        def emit_out_proj(n):
            for lc in range(nLC):
                op0 = big_ps.tile([128, E], F32, tag="big")
                for j in range(E // 512):
                    nc.tensor.matmul(
                        op0[:, j * 512 : (j + 1) * 512],
                        AT[:, n, lc * 128 : (lc + 1) * 128],
                        wcT_t[:, j * 512 : (j + 1) * 512],
                        start=True,
                        stop=True,
                    )
                ot = out_pool.tile([128, E], F32, tag="ot")
                nc.scalar.activation(ot[:], op0[:], COPY)
                nc.sync.dma_start(outp_v[lc * 128 : (lc + 1) * 128, n, :], ot[:])

        # --- phase C: per (batch, head) attention ---
        for p in range(NB * HPC):
            n, hh = p // HPC, p % HPC
            qh = qTs[hh * D : (hh + 1) * D, n, :]
            kh = kTs[hh * D : (hh + 1) * D, n, :]
            for lg in range(L // 256):
                evTs_t = evt_pool.tile([128, nSC, 256], BF16, tag="evts")
                ev_halves = []
                for half in range(2):
                    lc = lg * 2 + half
                    ev_t = ev_pool.tile([128, S], F32, tag="ev")
                    sums = small.tile([128, nW], F32, tag="sums")
                    for w in range(nW):
                        st = big_ps.tile([128, W], F32, tag="big")
                        for j in range(max(1, W // 512)):
                            jw = min(512, W)
                            nc.tensor.matmul(
                                st[:, j * jw : (j + 1) * jw],
                                qh[:, lc * 128 : (lc + 1) * 128],
                                kh[:, w * W + j * jw : w * W + (j + 1) * jw],
                                start=True,
                                stop=True,
                            )
                        # exp straight off PSUM (scores/8 <= ~6, no overflow);
                        # row max folded in later as e^-m = 1/rowmax
                        nc.scalar.activation(
                            ev_t[:, w * W : (w + 1) * W],
                            st[:],
                            EXP,
                            bias=0.0,
                            scale=0.125,
                            accum_out=sums[:, w : w + 1],
                        )
                    mx = small.tile([128, 1], F32, tag="mx")
                    nc.vector.reduce_max(mx[:], ev_t[:], axis=AX)
                    em = small.tile([128, 1], F32, tag="em")
                    nc.vector.reciprocal(em[:], mx[:])
                    stot = small.tile([128, 1], F32, tag="stot")
                    nc.vector.reduce_sum(stot[:], sums[:], axis=AX)
                    nc.gpsimd.tensor_tensor(stot[:], stot[:], em[:], op=MULT)
                    nc.gpsimd.tensor_scalar_add(stot[:], stot[:], float(S))
                    rr = small.tile([128, 1], F32, tag="rr")
                    nc.vector.reciprocal(rr[:], stot[:])
                    emr = small.tile([128, 1], F32, tag="emr")
                    nc.gpsimd.tensor_tensor(emr[:], em[:], rr[:], op=MULT)
                    nc.gpsimd.tensor_scalar_mul(
                        unc_t[:, p, lc : lc + 1], rr[:], float(S)
                    )
                    # bf16 attn for the transpose + attn @ V path (DVE, in
                    # parallel with the fp32 in-place pass on GpSimd below)
                    evb = evb_pool.tile([128, S], BF16, tag="evb")
                    nc.vector.tensor_scalar(evb[:], ev_t[:], emr[:], rr[:], MULT, ADD)
                    ev_halves.append(evb)
                    # attn = ev'*(e^-m * r) + r  (in place, fp32 for output)
                    nc.gpsimd.tensor_scalar(ev_t[:], ev_t[:], emr[:], rr[:], MULT, ADD)
                    nc.sync.dma_start(attn_o[p, lc * 128 : (lc + 1) * 128, :], ev_t[:])
                peg = 4 if nSC % 4 == 0 else (2 if nSC % 2 == 0 else 1)
                for sg in range(nSC // peg):
                    ets = ets_ps.tile([128, peg, 256], BF16, tag="etsbf")
                    for j in range(peg):
                        sc = sg * peg + j
                        for half in range(2):
                            nc.tensor.transpose(
                                ets[:, j, half * 128 : (half + 1) * 128],
                                ev_halves[half][:, sc * 128 : (sc + 1) * 128],
                                ident,
                            )
                    dst_sl = evTs_t[:, sg * peg : (sg + 1) * peg, :]
                    if sg % 2 == 0:
                        nc.vector.tensor_copy(dst_sl, ets[:])
                    else:
                        nc.scalar.activation(dst_sl, ets[:], COPY)
                avp = mid_ps.tile([128, 256], F32, tag="mid")
                for sc in range(nSC):
                    nc.tensor.matmul(
                        avp[:],
                        Vs[:, n, sc, :],
                        evTs_t[:, sc, :],
                        start=(sc == 0),
                        stop=(sc == nSC - 1),
                    )
                nc.scalar.activation(
                    AT[hh * D : (hh + 1) * D, n, lg * 256 : (lg + 1) * 256],
                    avp[hh * D : (hh + 1) * D, :],
                    COPY,
                )
        for n in range(NB):
            emit_out_proj(n)

        nc.sync.dma_start(unc_o[:], unc_t[:])

    nc.compile()
    return nc


_NC_CACHE = {}


def get_nc(L=2048, S=2048):
    key = (L, S)
    if key not in _NC_CACHE:
        _NC_CACHE[key] = build_nc(L, S)
    return _NC_CACHE[key]


def make_in_maps(query, key, value, in_proj_w, in_proj_b, out_proj_w):
    """Host-side sharding: slice per-core weights, flatten + bf16-cast
    activations."""
    L, N, Ei = query.shape
    S = key.shape[0]
    bf = ml_dtypes.bfloat16
    xq = np.ascontiguousarray(query.reshape(L * N, Ei)).astype(bf)
    xk = np.ascontiguousarray(key.reshape(S * N, Ei)).astype(bf)
    xv = np.ascontiguousarray(value.reshape(S * N, Ei)).astype(bf)
    in_maps = []
    for c in range(NCORES):
        r0 = c * D2
        wq = in_proj_w[r0 : r0 + D2, :]
        wk = in_proj_w[Ei + r0 : Ei + r0 + D2, :]
        wv = in_proj_w[2 * Ei + r0 : 2 * Ei + r0 + D2, :]
        in_maps.append(
            {
                "xq": xq,
                "xk": xk,
                "xv": xv,
                "wqT": np.ascontiguousarray(wq.T).astype(bf),
                "wkT": np.ascontiguousarray(wk.T).astype(bf),
                "wvT": np.ascontiguousarray(wv.T).astype(bf),
                "bq": np.ascontiguousarray(
                    in_proj_b[r0 : r0 + D2].reshape(D2, 1), dtype=np.float32
                ),
                "bk": np.ascontiguousarray(
                    in_proj_b[Ei + r0 : Ei + r0 + D2].reshape(D2, 1), dtype=np.float32
                ),
                "bv": np.ascontiguousarray(
                    in_proj_b[2 * Ei + r0 : 2 * Ei + r0 + D2].reshape(D2, 1),
                    dtype=np.float32,
                ),
                "wcT": np.ascontiguousarray(out_proj_w[:, r0 : r0 + D2].T).astype(
                    bf
                ),
            }
        )
    return in_maps


def assemble(results, out_proj_b, L, S):
    """Host-side gather: concat heads, sum out-proj partials, avg uncertainty."""
    nLC = L // 128
    attn_w = np.empty((NB, H, L, S), dtype=np.float32)
    avg_u = np.zeros((NB, L), dtype=np.float32)
    outp = np.zeros((L * NB, E), dtype=np.float32)
    for c, res in enumerate(results):
        a = res["attn"].reshape(NB, HPC, L, S)
        attn_w[:, c * HPC : (c + 1) * HPC] = a
        outp += res["outp"]
        # unc[p_inner, n*HPC+hh, lc] holds S*r for l = lc*128 + p_inner
        u = res["unc"].reshape(128, NB, HPC, nLC).sum(axis=2)  # (128, NB, nLC)
        avg_u += u.transpose(1, 2, 0).reshape(NB, L)
    avg_u /= H
    outp += out_proj_b[None, :].astype(np.float32)
    attn_out = outp.reshape(L, NB, E)
    return attn_out, attn_w, avg_u


def kernel(query, key, value, in_proj_w, in_proj_b, out_proj_w, out_proj_b):
    from concourse.bass_utils import run_bass_kernel_spmd

    L, N, Ei = query.shape
    S = key.shape[0]
    assert (N, Ei) == (NB, E)
    nc = get_nc(L, S)
    in_maps = make_in_maps(query, key, value, in_proj_w, in_proj_b, out_proj_w)
    res = run_bass_kernel_spmd(nc, in_maps, core_ids=list(range(NCORES)))
    return assemble(res.results, np.asarray(out_proj_b), L, S)
